# revision 95
# baseline (speedup 1.0000x reference)
"""Trainium2 Bass kernel for nn_Attention_36404142801494.

Fused causal self-attention (q=k=v=Wq(x)) + output projection, sharded over
8 NeuronCores: data-parallel on batch (B=2 -> 2 groups of 4 cores), tensor-
parallel on heads (8 heads -> 2 heads/core, dq = 128 hidden dims/core) with a
column-split Wq and a row-split Wo. Each core returns a partial [S, HID]
output (bf16); the host sums the 4 partials per batch and adds the Wo bias.

Per-core structure (keyed to the TimelineSim cost model, where a matmul costs
out_free_size x cycles_per_row and LDWEIGHTS is free):
  - qT [d=128, s] f32r via Wq matmuls (moving = x blocks, N=512/256).
  - QK emits scoresT tiles [k=128, q<=512] per key-chunk, causally trimmed;
    f32r keeps 1 cyc/row at N>=256 (diagonal chunks padded to N=256 min).
  - exp on ACT into bf16 et tiles; diagonal 128x128 blocks masked by a
    lower-triangular bf16 mask multiply on DVE (mask generated on device).
  - AV runs TRANSPOSED: stationary = et 128x128 block, moving = V chunk
    [k=128, 65] bf16 (col 64 = ones) -> av psum [q=128, 65], N=65/matmul.
    Col 64 accumulates the softmax denominator for free; normalization is a
    per-partition reciprocal + tensor_scalar multiply (no DRAM bounce).
  - ao [q, d] bf16 is PE-transposed (bf16 identity) to aoT [d, q]; both
    heads stack to [128, q] so Wo is one K=128 matmul per 128-query chunk
    (moving = WoT [128 d, 512 c] bf16, N=512).
  - Output partials DMA out as bf16 [2048, 512], split across HWDGE and
    SWDGE queues.

Schedule: unit (h, qb) = the QK/exp stream for one head x query block.
Units run (0,0),(1,0),(0,1),(1,1),(0,2),(0,3),(1,3),(1,2); qproj of the
next block is emitted as housekeeping inside an earlier unit so the ACT
engine keeps exp backlog across block boundaries.

Startup: wq (+bias col) and the first 128 query columns of x ride ONE DMA
(WqX) because HWDGE descriptor generation is a single serial unit (~625ns
per setup) — fewer setups gate the first qproj. woT rides SWDGE (Pool).
qb0's three qproj parts use separate psum pools: PSUM matmul WAR tracking
is coarse, so same-tile parts serialize behind each other's bias-add.
PE warmup transposes (garbage values, wap pool) beat the p-state ramp.

Drains: heads 1 of qb3/qb2 pre-accumulate their off-diagonal AV as ONE
psum accumulation group per block (PSUM allows only one open group per
2KB zero region — interleaved per-qc groups corrupt), closed at stream
end. Normalization is one reciprocal + ONE stride-0-broadcast
tensor_tensor over all 4 qc. Unit (1,2)'s masks are emitted ahead of
d3's final ob copies in the DVE queue. Tail output copies alternate
ACT/DVE; the two early output DMAs ride SWDGE so the late ones own the
serial HWDGE setup ladder.

Everything is hardcoded for B=2, S=2048, HID=512, NH=8, HD=64.
"""

import sys

sys.path.insert(0, "/opt/trn_rl_repo")

import numpy as np
import ml_dtypes

import concourse.bass as bass
import concourse.bacc as bacc
import concourse.tile as tile
import concourse.mybir as mybir
from concourse.bass_utils import run_bass_kernel_spmd

f32 = mybir.dt.float32
f32r = mybir.dt.float32r
bf16 = mybir.dt.bfloat16

B, S, HID = 2, 2048, 512
NH, HD = 8, 64
N_CORES = 8
SB = 512
SCALE = 1.0 / np.sqrt(HD)
N_WARMUP = 25
STREAM_PRIO = 160

Exp = mybir.ActivationFunctionType.Exp
ALU = mybir.AluOpType


def build_nc():
    nc = bacc.Bacc(None, target_bir_lowering=False)

    # host pre-arranged layouts (see make_in_maps):
    #   xB[p, i, s]  = x[b].T[128*i + p, s]
    #   WqX[p, :513] = Wq_w[dq, :].T row p ++ bias; [p, 513:] = xB[p, :, 0:128]
    #   WoT[d, c]    = Wo_w[:, dq].T[d, c]
    # wq and the first 128 query columns share one DMA (one HWDGE setup —
    # descriptor generation is a single serial unit, so fewer setups gate
    # the first qproj).
    xB = nc.dram_tensor("xB", [128, 4, S], bf16, kind="ExternalInput")
    WqX = nc.dram_tensor("WqX", [128, 1025], bf16, kind="ExternalInput")
    WoT = nc.dram_tensor("WoT", [128, HID], bf16, kind="ExternalInput")
    out_part = nc.dram_tensor("out_part", [S, HID], bf16, kind="ExternalOutput")

    with tile.TileContext(nc) as tc:
        with (
            tc.tile_pool(name="singles", bufs=1) as singles,
            tc.tile_pool(name="etp", bufs=15) as etp,
            tc.tile_pool(name="aop", bufs=6) as aop,
            tc.tile_pool(name="recp", bufs=2) as recp,
            tc.tile_pool(name="aotsb", bufs=4) as aotsb,
            tc.tile_pool(name="obp", bufs=4) as obp,
            tc.tile_pool(name="qkp", bufs=2, space="PSUM") as qkp,
            tc.tile_pool(name="ppp", bufs=1, space="PSUM") as ppp,
            tc.tile_pool(name="wap", bufs=2, space="PSUM") as wap,
            tc.tile_pool(name="avp", bufs=1, space="PSUM") as avp,
        ):
            # ---------------- prologue: DMAs + constants ----------------
            # PE warmup input: first gpsimd op (Pool frees at ~500ns), so
            # warm transposes launch by ~900ns and the p-state ramp finishes
            # during qproj(0)'s first part
            warm_in = singles.tile([64, 64], f32, tag="warm_in")
            nc.gpsimd.memset(warm_in, 1.0)

            # exp ACT table preload: no waits, so the 1283ns table load +
            # preload run first thing on the idle ACT engine
            preld = singles.tile([32, 32], f32, tag="preld")
            nc.scalar.activation(
                out=preld, in_=warm_in[0:32, 0:32], func=Exp, scale=1.0
            )

            # SP HWDGE queue in need-order (setups serialize at ~625ns each)
            wqx = singles.tile([128, 1025], bf16, tag="wqx")
            nc.sync.dma_start(out=wqx, in_=WqX[:, :])
            wq = wqx[:, 0:513]
            xs = singles.tile([128, 4, S], bf16, tag="xs")
            nc.sync.dma_start(out=xs[:, :, 128:256], in_=xB[:, :, 128:256])
            nc.sync.dma_start(out=xs[:, :, 256:512], in_=xB[:, :, 256:512])
            nc.sync.dma_start(out=xs[:, :, 512:1024], in_=xB[:, :, 512:1024])
            nc.sync.dma_start(out=xs[:, :, 1024:1536], in_=xB[:, :, 1024:1536])
            nc.sync.dma_start(out=xs[:, :, 1536:2048], in_=xB[:, :, 1536:2048])

            # dependency-free PE warmup (values irrelevant). Lives in the wap
            # pool (first real use ~6.5us) so qproj(0) never waits on it.
            warm = wap.tile([128, SB], f32, tag="wap", name="warm")
            for j in range(N_WARMUP):
                nc.tensor.transpose(
                    warm[0:64, 64 * (j % 8) : 64 * (j % 8) + 64],
                    warm_in, warm_in,
                )

            # tensor_scalar needs an f32 scalar operand: unpack the bias col
            wqbf = singles.tile([128, 1], f32, tag="wqbf")
            nc.vector.tensor_copy(wqbf, wq[:, 512:513])

            identf = singles.tile([128, 64], f32, tag="identf")
            nc.gpsimd.memset(identf, 1.0)
            for p0 in (0, 64):
                nc.gpsimd.affine_select(
                    out=identf[p0 : p0 + 64, :], in_=identf[p0 : p0 + 64, :],
                    compare_op=ALU.is_equal,
                    fill=0.0, base=0, pattern=[[-1, 64]], channel_multiplier=1,
                )

            ident64 = singles.tile([128, 64], bf16, tag="ident64")
            nc.vector.tensor_copy(ident64, identf)
            identb = singles.tile([128, 128], bf16, tag="identb")
            nc.gpsimd.memset(identb, 1.0)
            nc.gpsimd.affine_select(
                out=identb, in_=identb, compare_op=ALU.is_equal,
                fill=0.0, base=0, pattern=[[-1, 128]], channel_multiplier=1,
            )
            # trib[k, q] = 1 if k <= q else 0   (iota = q - k >= 0)
            trib = singles.tile([128, 128], bf16, tag="trib")
            nc.gpsimd.memset(trib, 1.0)
            nc.gpsimd.affine_select(
                out=trib, in_=trib, compare_op=ALU.is_ge,
                fill=0.0, base=0, pattern=[[1, 128]], channel_multiplier=-1,
            )

            qT = singles.tile([128, S], bf16, tag="qT")
            v_sb = [
                singles.tile([128, 16, 65], bf16, tag=f"v{h}", name=f"v{h}")
                for h in range(2)
            ]
            for h in range(2):
                nc.gpsimd.memset(v_sb[h][:, :, 64:65], 1.0)

            # SWDGE (Pool) carries woT: needed only by the first Wo (~12us).
            # Emitted after the Pool constants so its transfer doesn't jump
            # ahead of the early x chunks on the shared DMA pipe.
            woT = singles.tile([128, HID], bf16, tag="woT")
            nc.gpsimd.dma_start(out=woT, in_=WoT[:, :])

            # state shared across the emission helpers
            et_map = {}    # (h, qb, kc) -> (et_tile, col_of_qc0)
            pending_masks = {}  # (h, qb, qc_local) -> (et_tile, col)
            ao_tiles = {}  # (h, qc_local) -> ao tile (bf16 [128, 64])
            av_cur = {}    # h -> av psum tile
            aot_ps = {}    # (h, qb) -> psum tile [64, 4, 128] bf16
            aot_sb = {}    # qb -> sbuf tile [128, 4, 128] bf16

            # ---------------- emission helpers ----------------
            def qproj(qb, halves=False):
                s0 = qb * SB
                if halves:
                    # PSUM matmul WAR tracking is coarse: a part's matmuls
                    # wait on the previous parts' bias-add reads if they
                    # share a tile. Separate tiles/pools let the three
                    # DMA-gated parts of qb0 overlap.
                    parts = (
                        (0, 128, ppp.tile([128, 128], f32, tag="pp", name="qp0a")),
                        (128, 256, wap.tile([128, 128], f32, tag="wap", name="qp0b")),
                        (256, 512, ppp.tile([128, 256], f32, tag="pp", name="qp0c")),
                    )
                else:
                    parts = ((0, SB, ppp.tile([128, SB], f32, tag="pp", name=f"qp{qb}")),)
                for c0, c1, qp in parts:
                    for i in range(4):
                        if qb == 0 and c1 <= 128:
                            # cols 0:128 rode in with wq (one packed DMA)
                            xsrc = wqx[:, 513 + 128 * i : 513 + 128 * i + 128]
                        else:
                            xsrc = xs[:, i, s0 + c0 : s0 + c1]
                        nc.tensor.matmul(
                            qp[:, 0 : c1 - c0], lhsT=wq[:, 128 * i : 128 * i + 128],
                            rhs=xsrc,
                            start=(i == 0), stop=(i == 3),
                        )
                    nc.vector.tensor_scalar_add(
                        qT[:, s0 + c0 : s0 + c1], qp[:, 0 : c1 - c0], wqbf
                    )


            def vprep(h, qb):
                hp = 64 * h
                vt = ppp.tile([128, 4, 64], bf16, tag="pp", name=f"vt{h}_{qb}")
                for j in range(4):
                    t0 = 128 * (4 * qb + j)
                    nc.tensor.transpose(
                        vt[:, j, :], qT[hp : hp + 64, t0 : t0 + 128],
                        ident64[hp : hp + 64, :],
                    )
                nc.vector.tensor_copy(v_sb[h][:, 4 * qb : 4 * qb + 4, 0:64], vt)

            def qk_group(h, qb, chunks, expw, masks, exp_splits=None,
                         defer_masks=True, small=False):
                """chunks: [(kc, coff, qoff, N)]; masks: [col] of tri blocks.
                small=True puts the narrow diag-1 scores in a wap-pool bank:
                it leaves the qkp pingpong, so diag-2 and the next unit's
                first QK wait only long-finished exps."""
                hp = 64 * h
                s0 = qb * SB
                if small:
                    qk = wap.tile([128, SB], f32, tag="wap", name="qkS")
                else:
                    qk = qkp.tile([128, 1024], f32, tag="qk", name="qk")
                et = etp.tile([128, 1024], bf16, tag="et", name="et")
                ranges = exp_splits or [(0, expw)]
                for kc, coff, qoff, n in chunks:
                    t0 = 128 * kc
                    nc.tensor.matmul(
                        qk[:, coff : coff + n],
                        lhsT=qT[hp : hp + 64, t0 : t0 + 128],
                        rhs=qT[hp : hp + 64, s0 + qoff : s0 + qoff + n],
                        start=True, stop=True,
                    )
                    et_map[(h, qb, kc)] = (et, coff - 128 * (qoff // 128))
                    # exp as soon as the covering chunk(s) are in psum
                    while ranges and ranges[0][1] <= coff + n:
                        e0, e1 = ranges.pop(0)
                        nc.scalar.activation(
                            out=et[:, e0:e1], in_=qk[:, e0:e1],
                            func=Exp, scale=SCALE,
                        )
                for e0, e1 in ranges:
                    nc.scalar.activation(
                        out=et[:, e0:e1], in_=qk[:, e0:e1], func=Exp, scale=SCALE
                    )
                for qc_l, mc in masks:
                    if defer_masks:
                        pending_masks[(h, qb, qc_l)] = (et, mc)
                    else:
                        # final unit: Pool is idle and this keeps the DVE
                        # queue clear for the drain chains' recip/mul
                        nc.gpsimd.tensor_mul(
                            et[:, mc : mc + 128], et[:, mc : mc + 128], trib
                        )

            def unit_groups(h, qb, split_first=False, defer_masks=True,
                            diag_pos=None):
                k0 = 4 * qb
                gs = []
                for ke in range(0, k0, 2):  # off-diagonal pairs, full width
                    gs.append(
                        lambda ke=ke: qk_group(
                            h, qb,
                            [(ke, 0, 0, 512), (ke + 1, 512, 0, 512)],
                            1024, [],
                        )
                    )
                ch0 = (
                    [(k0, 0, 0, 128), (k0, 128, 128, 128), (k0, 256, 256, 256)]
                    if split_first else [(k0, 0, 0, 512)]
                )
                splits = [(0, 128), (128, 256), (256, 512)] if split_first else None
                diags = [
                    # diag pack A: kc0 alone (N=512). The NARROW exp comes
                    # first so the next unit's first QK (pingpong on this
                    # tile) unblocks early and overlaps pack B's wider exp:
                    # no ACT gap at unit boundaries.
                    lambda: qk_group(
                        h, qb, ch0, 512, [(0, 0)],
                        exp_splits=splits, defer_masks=defer_masks,
                        # the drain unit (diag_pos set) keeps qkp: no next
                        # unit to unblock, and its wap rotation is busy with
                        # d3's drain tiles
                        small=not split_first and diag_pos is None,
                    ),
                    # diag pack B: kc1 (384) + kc3 (128) + kc2 (256), packed
                    # 0..768 contiguously without crossing a psum bank
                    lambda: qk_group(
                        h, qb,
                        [(k0 + 1, 0, 128, 384), (k0 + 3, 384, 384, 128),
                         (k0 + 2, 512, 256, 256)],
                        768, [(1, 0), (2, 512), (3, 384)],
                        defer_masks=defer_masks,
                    ),
                ]
                if diag_pos is None:
                    return gs + diags
                return gs[:diag_pos] + diags + gs[diag_pos:]

            def av_item(h, qb, qc_local, kc_from=0, kc_to=None):
                qc = 4 * qb + qc_local
                if kc_to is None:
                    kc_to = qc + 1
                if qc_local == 0 and kc_from == 0:
                    av_cur[h] = avp.tile(
                        [128, 4, 65], f32, tag="av", name=f"av{h}{qb}"
                    )
                av = av_cur[h]
                if kc_to > qc:  # this call includes the diagonal chunk
                    pm = pending_masks.pop((h, qb, qc_local), None)
                    if pm is not None:
                        met, mc = pm
                        nc.vector.tensor_mul(
                            met[:, mc : mc + 128], met[:, mc : mc + 128], trib
                        )
                for kc in range(kc_from, min(kc_to, qc + 1)):
                    et, c0 = et_map[(h, qb, kc)]
                    nc.tensor.matmul(
                        av[:, qc_local, :],
                        lhsT=et[:, c0 + 128 * qc_local : c0 + 128 * qc_local + 128],
                        rhs=v_sb[h][:, kc, :],
                        start=(kc == 0), stop=(kc == qc),
                    )

            def av_pre_step(h, qb, kcs, stop_at_end=False):
                """kc-major partial AV accumulation for a drain block. PSUM
                allows ONE open accumulation group per 2KB zero region (the
                whole av tile), so the drain's entire AV is a single group:
                started here at kc==0, closed by av_diag(last=True) or by
                stop_at_end on the final chunk."""
                if kcs and kcs[0] == 0:
                    av_cur[h] = avp.tile(
                        [128, 4, 65], f32, tag="av", name=f"av{h}{qb}"
                    )
                av = av_cur[h]
                for kc in kcs:
                    et, c0 = et_map[(h, qb, kc)]
                    for qc_local in range(4):
                        nc.tensor.matmul(
                            av[:, qc_local, :],
                            lhsT=et[:, c0 + 128 * qc_local : c0 + 128 * qc_local + 128],
                            rhs=v_sb[h][:, kc, :],
                            start=(kc == 0 and qc_local == 0),
                            stop=(stop_at_end and kc == kcs[-1] and qc_local == 3),
                        )

            def av_diag(h, qb, qc_local, last):
                """Diagonal AV chunks of one drain chain; continues the open
                group from av_pre_step, stop only on the unit's final mm."""
                qc = 4 * qb + qc_local
                av = av_cur[h]
                pm = pending_masks.pop((h, qb, qc_local), None)
                if pm is not None:
                    met, mc = pm
                    nc.vector.tensor_mul(
                        met[:, mc : mc + 128], met[:, mc : mc + 128], trib
                    )
                for kc in range(4 * qb, qc + 1):
                    et, c0 = et_map[(h, qb, kc)]
                    nc.tensor.matmul(
                        av[:, qc_local, :],
                        lhsT=et[:, c0 + 128 * qc_local : c0 + 128 * qc_local + 128],
                        rhs=v_sb[h][:, kc, :],
                        start=False, stop=(last and kc == qc),
                    )

            def div_item(h, qc_local, rec=None):
                """ao = av[:, qc, 0:64] * (1 / av[:, qc, 64]) -> bf16."""
                av = av_cur[h]
                if rec is None:  # per-qc reciprocal (drain path)
                    rec = recp.tile([128, 1, 1], f32, tag="rec", name="rec1")
                    nc.vector.reciprocal(rec, av[:, qc_local, 64:65])
                    rslice = rec[:, 0, :]
                else:
                    rslice = rec[:, qc_local, :]
                ao = aop.tile([128, 64], bf16, tag="ao", name="ao")
                nc.vector.tensor_scalar_mul(ao, av[:, qc_local, 0:64], rslice)
                ao_tiles[(h, qc_local)] = ao

            def norm_item(h, qb):
                rec = recp.tile([128, 4, 1], f32, tag="rec", name="rec4")
                nc.vector.reciprocal(rec, av_cur[h][:, :, 64:65])
                # all 4 qc normalized in ONE DVE op via a stride-0 broadcast
                # of the per-(partition, qc) reciprocal
                ao4 = aop.tile([128, 4, 64], bf16, tag="ao4", name="ao4")
                a_ap, r_ap = bass.broadcast_tensor_aps(
                    av_cur[h][:, :, 0:64], rec[:, :, :]
                )
                nc.vector.tensor_tensor(out=ao4, in0=a_ap, in1=r_ap, op=ALU.mult)
                for qc_local in range(4):
                    ao_tiles[(h, qc_local)] = ao4[:, qc_local, :]

            def t_item(h, qb):
                ps = wap.tile([64, 4, 128], bf16, tag="wap", name=f"aot{h}{qb}")
                aot_ps[(h, qb)] = ps
                for qc_local in range(4):
                    nc.tensor.transpose(
                        ps[:, qc_local, :], ao_tiles[(h, qc_local)], identb
                    )

            def aot_copy(qb, h):
                if qb not in aot_sb:
                    aot_sb[qb] = aotsb.tile(
                        [128, 4, 128], bf16, tag="aotsb", name=f"aotsb{qb}"
                    )
                sb = aot_sb[qb]
                nc.vector.tensor_copy(
                    sb[64 * h : 64 * h + 64, :, :], aot_ps[(h, qb)]
                )

            def w_item(qb, qc_local, copy_eng, dma_eng, wp=None):
                if wp is None:
                    wp = wap.tile(
                        [128, SB], f32, tag="wap", name=f"wp{qb}{qc_local}"
                    )
                nc.tensor.matmul(
                    wp, lhsT=aot_sb[qb][:, qc_local, :], rhs=woT,
                    start=True, stop=True,
                )
                ob = obp.tile([128, SB], bf16, tag="ob", name="ob")
                if hasattr(copy_eng, "tensor_copy"):
                    copy_eng.tensor_copy(ob, wp)
                else:
                    copy_eng.copy(ob, wp)  # scalar engine (ACT)
                r0 = 512 * qb + 128 * qc_local
                dma_eng.dma_start(out=out_part[r0 : r0 + 128, :], in_=ob)

            def w_items(qb):
                # ob copies must read PSUM: only DVE/ACT can. DMAs alternate
                # between the HWDGE (sync) and SWDGE (gpsimd) queues.
                out = []
                for qc_local in range(4):
                    out.append(lambda q=qb, c=qc_local: w_item(q, c, nc.vector, nc.sync))
                return out

            def av_norm_t(h, qb):
                return [lambda c=c: av_item(h, qb, c) for c in range(4)] + [
                    lambda: norm_item(h, qb),
                    lambda: t_item(h, qb),
                ]

            def drain_steps(qb, engines, use_qkp=False, pre=False):
                """Software-pipelined drain of head 1 of block qb: the four
                per-qc chains (AV -> divide -> transpose -> copy -> Wo -> DMA)
                emitted as a diagonal wavefront so the in-order engines never
                wait a full chain. Returns a list of emit-thunks (steps); with
                pre=True the off-diagonal AV accumulation is split out as four
                leading steps that only need the unit's off-diagonal exps."""
                h = 1
                ps_t = {}

                def av_s(qc):
                    if not pre:
                        return lambda: av_item(h, qb, qc)
                    return lambda: av_diag(h, qb, qc, last=(qc == 3))

                def div_s(qc):
                    return lambda: div_item(h, qc)

                def t_s(qc):
                    def f():
                        ps = wap.tile(
                            [64, 1, 128], bf16, tag="wap", name=f"aotd{qb}{qc}"
                        )
                        ps_t[qc] = ps
                        nc.tensor.transpose(
                            ps[:, 0, :], ao_tiles[(h, qc)], identb
                        )
                    return f

                def cp_s(qc):
                    def f():
                        nc.vector.tensor_copy(
                            aot_sb[qb][64:128, qc, :], ps_t[qc][:, 0, :]
                        )
                    return f

                def w_s(qc, use_qkp):
                    ce, de = engines[qc]

                    def f():
                        wp = None
                        if use_qkp:
                            # the QK stream is done: its psum banks are free.
                            # One tile per Wo — sharing a [128,1024] tile
                            # between two Wos serializes the second behind
                            # the first's output copy (coarse matmul WAR).
                            wp = qkp.tile(
                                [128, 1024], f32, tag="qk", name=f"wpd{qc}"
                            )[:, 0:512]
                        w_item(qb, qc, ce, de, wp=wp)

                    return f

                uq = use_qkp
                if pre:
                    # batched variant: diag AVs, then ONE broadcast norm for
                    # all 4 qc, then transposes/copies/Wos wavefronted
                    waves = [
                        [av_s(0), av_s(1)],
                        [av_s(2), av_s(3)],
                        [lambda: norm_item(h, qb)],
                        [t_s(0), t_s(1)],
                        [t_s(2), t_s(3), cp_s(0)],
                        [cp_s(1), cp_s(2), cp_s(3)],
                        [w_s(0, uq), w_s(1, uq)],
                        [w_s(2, uq), w_s(3, uq)],
                    ]
                else:
                    waves = [
                        [av_s(0)],
                        [av_s(1), div_s(0)],
                        [av_s(2), div_s(1), t_s(0)],
                        [av_s(3), div_s(2), cp_s(0), t_s(1)],
                        [div_s(3), cp_s(1), t_s(2), w_s(0, uq)],
                        [cp_s(2), t_s(3), w_s(1, uq)],
                        [cp_s(3), w_s(2, uq)],
                        [w_s(3, uq)],
                    ]

                def run(wave):
                    return lambda: [f() for f in wave]

                steps = [run(w) for w in waves]
                if pre:
                    nk = 4 * qb
                    chunks = [
                        list(range(nk * i // 4, nk * (i + 1) // 4))
                        for i in range(4)
                    ]
                    steps = [
                        lambda ks=ks: av_pre_step(h, qb, ks) for ks in chunks
                    ] + steps
                return steps

            def emit_unit(h, qb, hk, split_first=False, defer_masks=True,
                          diag_pos=None, skip_first=0):
                gs = unit_groups(h, qb, split_first=split_first,
                                 defer_masks=defer_masks, diag_pos=diag_pos)
                gs = gs[skip_first:]
                hk = list(hk)
                for g in gs:
                    g()
                    if hk:
                        hk.pop(0)()
                for item in hk:
                    item()
                return []


            def interleave(ws, avs, rest):
                """Alternate stall-prone W chains with cheap AV filler so an
                in-order PE never has two wp-waits back to back."""
                out = []
                for i in range(max(len(ws), len(avs))):
                    if i < len(ws):
                        out.append(ws[i])
                    if i < len(avs):
                        out.append(avs[i])
                return out + rest

            # ---------------- main schedule ----------------
            # unit order: (0,0) (1,0) (0,1) (1,1) (0,2) (0,3) (1,3)
            #             (1,2)+[qb3 drain] [qb2 drain]
            qproj(0, halves=True)
            emit_unit(
                0, 0, [lambda: vprep(0, 0), lambda: vprep(1, 0)],
                split_first=True,
            )
            emit_unit(
                1, 0,
                [lambda: qproj(1)] + av_norm_t(0, 0)
                + [lambda: aot_copy(0, 0)],
            )
            emit_unit(
                0, 1,
                [lambda: qproj(2)] + av_norm_t(1, 0)
                + [lambda: aot_copy(0, 1),
                   lambda: vprep(0, 1), lambda: vprep(1, 1)],
            )
            emit_unit(
                1, 1,
                [lambda: qproj(3)]
                + interleave(
                    w_items(0),
                    [lambda c=c: av_item(0, 1, c) for c in range(4)],
                    [lambda: norm_item(0, 1), lambda: t_item(0, 1),
                     lambda: aot_copy(1, 0)],
                ),
            )
            emit_unit(
                0, 2,
                av_norm_t(1, 1)
                + [lambda: aot_copy(1, 1),
                   lambda: vprep(0, 2), lambda: vprep(1, 2)],
            )
            emit_unit(
                0, 3,
                interleave(
                    w_items(1),
                    [lambda c=c: av_item(0, 2, c) for c in range(4)],
                    [lambda: norm_item(0, 2), lambda: t_item(0, 2),
                     lambda: aot_copy(2, 0),
                     lambda: vprep(0, 3), lambda: vprep(1, 3)],
                ),
            )
            # qb3 drain: off-diag AV pre-accumulates inside unit (1,3); the
            # per-qc chains (diag AV -> div -> t -> cp -> Wo -> ob -> DMA)
            # ride unit (1,2)'s off-diag groups. Output DMAs go out SWDGE
            # (Pool gen) so the HWDGE unit stays clear for the final drain.
            d3 = drain_steps(3, {c: (nc.vector, nc.gpsimd) for c in range(4)},
                             pre=True)
            emit_unit(
                1, 3,
                av_norm_t(0, 3) + [lambda: aot_copy(3, 0)] + d3[:4],
            )
            # qb2 drain: unit (1,2) runs with its diagonal packs mid-unit
            # (off01, off23, diagA, diagB, off45, off67) so the masks and
            # diag AV land mid-stream. The single AV accumulation group
            # closes on the last off-diag chunk right at stream end, so all
            # four chains launch together.
            w3 = d3[4:]
            pair = lambda a, b: (lambda: (a(), b()))

            def d2_masks():
                # pop + apply all four masks ahead of d3's final ob copies in
                # the DVE queue, so the diag AVs don't stall on a 658ns copy
                for qc in range(4):
                    pm = pending_masks.pop((1, 2, qc), None)
                    if pm is not None:
                        met, mc = pm
                        nc.vector.tensor_mul(
                            met[:, mc : mc + 128], met[:, mc : mc + 128], trib
                        )

            emit_unit(
                1, 2,
                [pair(w3[0], w3[1]), pair(w3[2], w3[3]),
                 pair(w3[4], w3[5]),
                 pair(d2_masks, pair(w3[6], w3[7])),
                 # av12 allocation must follow ALL d3 wave emissions (avp is
                 # a single rotating bank shared with av13)
                 pair(lambda: av_pre_step(1, 2, [0, 1, 2, 3, 4, 5]),
                      lambda: [av_diag(1, 2, c, last=False) for c in range(4)]),
                 lambda: av_pre_step(1, 2, [6, 7], stop_at_end=True)],
                diag_pos=2,
            )
            # post-stream chains: one reciprocal for all 4 qc; aot copies
            # split DVE/ACT; Wo psum spread over four pools (no bank reuse
            # stalls); the last two outputs DMA straight from psum as f32
            # (host converts), the first two copy to bf16 on ACT/DVE.
            rec4 = recp.tile([128, 4, 1], f32, tag="rec", name="rec4d")
            nc.vector.reciprocal(rec4, av_cur[1][:, :, 64:65])
            ao4d = aop.tile([128, 4, 64], bf16, tag="ao4", name="ao4d")
            a_ap, r_ap = bass.broadcast_tensor_aps(
                av_cur[1][:, :, 0:64], rec4[:, :, :]
            )
            nc.vector.tensor_tensor(out=ao4d, in0=a_ap, in1=r_ap, op=ALU.mult)
            ps_t2 = {}
            for qc in range(4):
                ps = wap.tile([64, 1, 128], bf16, tag="wap", name=f"aotd2{qc}")
                ps_t2[qc] = ps
                nc.tensor.transpose(ps[:, 0, :], ao4d[:, qc, :], identb)
            for qc, ce in ((0, nc.vector), (1, nc.scalar),
                           (2, nc.vector), (3, nc.scalar)):
                if hasattr(ce, "tensor_copy"):
                    ce.tensor_copy(aot_sb[2][64:128, qc, :], ps_t2[qc][:, 0, :])
                else:
                    ce.copy(aot_sb[2][64:128, qc, :], ps_t2[qc][:, 0, :])
            wps = {}
            for qc in range(4):
                if qc < 2:
                    wp = qkp.tile([128, 1024], f32, tag="qk",
                                  name=f"wpd{qc}")[:, 0:512]
                elif qc == 2:
                    wp = ppp.tile([128, SB], f32, tag="pp", name="wpd2")
                else:
                    wp = wap.tile([128, SB], f32, tag="wap", name="wpd3")
                nc.tensor.matmul(wp, lhsT=aot_sb[2][:, qc, :], rhs=woT,
                                 start=True, stop=True)
                wps[qc] = wp
            # output copies: qc0 full on ACT, qc1 full on DVE; the two LAST
            # chains split their copies half/half across ACT+DVE (~350ns
            # each) so the final DMA launches ~0.6us sooner. Early outputs
            # ride SWDGE (Pool gen, parallel to the HWDGE unit).
            for qc, ce, de in ((0, nc.scalar, nc.gpsimd),
                               (1, nc.vector, nc.gpsimd),
                               (2, nc.scalar, nc.sync),
                               (3, nc.vector, nc.sync)):
                ob = obp.tile([128, SB], bf16, tag="ob", name=f"obd{qc}")
                if hasattr(ce, "tensor_copy"):
                    ce.tensor_copy(ob, wps[qc])
                else:
                    ce.copy(ob, wps[qc])
                r0 = 1024 + 128 * qc
                de.dma_start(out=out_part[r0 : r0 + 128, :], in_=ob)

    nc.finalize()
    return nc


_NC_CACHE = None


def _get_nc():
    global _NC_CACHE
    if _NC_CACHE is None:
        _NC_CACHE = build_nc()
    return _NC_CACHE


def make_in_maps(x, Wq_w, Wq_b, Wo_w):
    x = np.asarray(x, dtype=np.float32)
    Wq_w = np.asarray(Wq_w, dtype=np.float32)
    Wq_b = np.asarray(Wq_b, dtype=np.float32)
    Wo_w = np.asarray(Wo_w, dtype=np.float32)
    in_maps = []
    for c in range(N_CORES):
        b, hp = divmod(c, 4)
        dq = slice(128 * hp, 128 * (hp + 1))
        xBc = np.ascontiguousarray(x[b].T.reshape(4, 128, S).transpose(1, 0, 2))
        WqBc = np.ascontiguousarray(
            Wq_w[dq, :].T.reshape(4, 128, 128).transpose(1, 0, 2)
        )
        WqBp = np.concatenate(
            [
                WqBc.reshape(128, 512),
                Wq_b[dq].reshape(128, 1),
                # first 128 query columns of each i-block ride with wq
                xBc[:, :, 0:128].reshape(128, 512),
            ],
            axis=1,
        )
        in_maps.append({
            "xB": xBc.astype(ml_dtypes.bfloat16),
            "WqX": np.ascontiguousarray(WqBp).astype(ml_dtypes.bfloat16),
            "WoT": np.ascontiguousarray(Wo_w[:, dq].T).astype(ml_dtypes.bfloat16),
        })
    return in_maps


def kernel(x, mask, Wq_w, Wq_b, Wo_w, Wo_b, **_):
    nc = _get_nc()
    in_maps = make_in_maps(x, Wq_w, Wq_b, Wo_w)
    res = run_bass_kernel_spmd(nc, in_maps, core_ids=list(range(N_CORES)))
    Wo_b = np.asarray(Wo_b, dtype=np.float32)
    out = np.empty((B, S, HID), dtype=np.float32)
    for b in range(B):
        acc = np.asarray(res.results[4 * b]["out_part"], dtype=np.float32)
        for c in range(4 * b + 1, 4 * b + 4):
            acc = acc + np.asarray(res.results[c]["out_part"], dtype=np.float32)
        out[b] = acc + Wo_b[None, :]
    return out



# revision 96
# speedup vs baseline: 1.0046x; 1.0046x over previous
"""Trainium2 Bass kernel for nn_Attention_36404142801494.

Fused causal self-attention (q=k=v=Wq(x)) + output projection, sharded over
8 NeuronCores: data-parallel on batch (B=2 -> 2 groups of 4 cores), tensor-
parallel on heads (8 heads -> 2 heads/core, dq = 128 hidden dims/core) with a
column-split Wq and a row-split Wo. Each core returns a partial [S, HID]
output (bf16); the host sums the 4 partials per batch and adds the Wo bias.

Per-core structure (keyed to the TimelineSim cost model, where a matmul costs
out_free_size x cycles_per_row and LDWEIGHTS is free):
  - qT [d=128, s] f32r via Wq matmuls (moving = x blocks, N=512/256).
  - QK emits scoresT tiles [k=128, q<=512] per key-chunk, causally trimmed;
    f32r keeps 1 cyc/row at N>=256 (diagonal chunks padded to N=256 min).
  - exp on ACT into bf16 et tiles; diagonal 128x128 blocks masked by a
    lower-triangular bf16 mask multiply on DVE (mask generated on device).
  - AV runs TRANSPOSED: stationary = et 128x128 block, moving = V chunk
    [k=128, 65] bf16 (col 64 = ones) -> av psum [q=128, 65], N=65/matmul.
    Col 64 accumulates the softmax denominator for free; normalization is a
    per-partition reciprocal + tensor_scalar multiply (no DRAM bounce).
  - ao [q, d] bf16 is PE-transposed (bf16 identity) to aoT [d, q]; both
    heads stack to [128, q] so Wo is one K=128 matmul per 128-query chunk
    (moving = WoT [128 d, 512 c] bf16, N=512).
  - Output partials DMA out as bf16 [2048, 512], split across HWDGE and
    SWDGE queues.

Schedule: unit (h, qb) = the QK/exp stream for one head x query block.
Units run (0,0),(1,0),(0,1),(1,1),(0,2),(0,3),(1,3),(1,2); qproj of the
next block is emitted as housekeeping inside an earlier unit so the ACT
engine keeps exp backlog across block boundaries.

Startup: wq (+bias col) and the first 128 query columns of x ride ONE DMA
(WqX) because HWDGE descriptor generation is a single serial unit (~625ns
per setup) — fewer setups gate the first qproj. woT rides SWDGE (Pool).
qb0's three qproj parts use separate psum pools: PSUM matmul WAR tracking
is coarse, so same-tile parts serialize behind each other's bias-add.
PE warmup transposes (garbage values, wap pool) beat the p-state ramp.

Drains: heads 1 of qb3/qb2 pre-accumulate their off-diagonal AV as ONE
psum accumulation group per block (PSUM allows only one open group per
2KB zero region — interleaved per-qc groups corrupt), closed at stream
end. Normalization is one reciprocal + ONE stride-0-broadcast
tensor_tensor over all 4 qc. Unit (1,2)'s masks are emitted ahead of
d3's final ob copies in the DVE queue. Tail output copies alternate
ACT/DVE; the two early output DMAs ride SWDGE so the late ones own the
serial HWDGE setup ladder.

Everything is hardcoded for B=2, S=2048, HID=512, NH=8, HD=64.
"""

import sys

sys.path.insert(0, "/opt/trn_rl_repo")

import numpy as np
import ml_dtypes

import concourse.bass as bass
import concourse.bacc as bacc
import concourse.tile as tile
import concourse.mybir as mybir
from concourse.bass_utils import run_bass_kernel_spmd

f32 = mybir.dt.float32
f32r = mybir.dt.float32r
bf16 = mybir.dt.bfloat16

B, S, HID = 2, 2048, 512
NH, HD = 8, 64
N_CORES = 8
SB = 512
SCALE = 1.0 / np.sqrt(HD)
N_WARMUP = 25
STREAM_PRIO = 160

Exp = mybir.ActivationFunctionType.Exp
ALU = mybir.AluOpType


def build_nc():
    nc = bacc.Bacc(None, target_bir_lowering=False)

    # host pre-arranged layouts (see make_in_maps):
    #   xB[p, i, s]  = x[b].T[128*i + p, s]
    #   WqX[p, :513] = Wq_w[dq, :].T row p ++ bias; [p, 513:] = xB[p, :, 0:128]
    #   WoT[d, c]    = Wo_w[:, dq].T[d, c]
    # wq and the first 128 query columns share one DMA (one HWDGE setup —
    # descriptor generation is a single serial unit, so fewer setups gate
    # the first qproj).
    xB = nc.dram_tensor("xB", [128, 4, S], bf16, kind="ExternalInput")
    WqX = nc.dram_tensor("WqX", [128, 1025], bf16, kind="ExternalInput")
    WoT = nc.dram_tensor("WoT", [128, HID], bf16, kind="ExternalInput")
    out_part = nc.dram_tensor("out_part", [S, HID], bf16, kind="ExternalOutput")

    with tile.TileContext(nc) as tc:
        with (
            tc.tile_pool(name="singles", bufs=1) as singles,
            tc.tile_pool(name="etp", bufs=15) as etp,
            tc.tile_pool(name="aop", bufs=6) as aop,
            tc.tile_pool(name="recp", bufs=2) as recp,
            tc.tile_pool(name="aotsb", bufs=4) as aotsb,
            tc.tile_pool(name="obp", bufs=4) as obp,
            tc.tile_pool(name="qkp", bufs=2, space="PSUM") as qkp,
            tc.tile_pool(name="ppp", bufs=1, space="PSUM") as ppp,
            tc.tile_pool(name="wap", bufs=2, space="PSUM") as wap,
            tc.tile_pool(name="avp", bufs=1, space="PSUM") as avp,
        ):
            # ---------------- prologue: DMAs + constants ----------------
            # PE warmup input: first gpsimd op (Pool frees at ~500ns), so
            # warm transposes launch by ~900ns and the p-state ramp finishes
            # during qproj(0)'s first part
            warm_in = singles.tile([64, 64], f32, tag="warm_in")
            nc.gpsimd.memset(warm_in, 1.0)

            # exp ACT table preload: no waits, so the 1283ns table load +
            # preload run first thing on the idle ACT engine
            preld = singles.tile([32, 32], f32, tag="preld")
            nc.scalar.activation(
                out=preld, in_=warm_in[0:32, 0:32], func=Exp, scale=1.0
            )

            # SP HWDGE queue in need-order (setups serialize at ~625ns each)
            wqx = singles.tile([128, 1025], bf16, tag="wqx")
            nc.sync.dma_start(out=wqx, in_=WqX[:, :])
            wq = wqx[:, 0:513]
            xs = singles.tile([128, 4, S], bf16, tag="xs")
            nc.sync.dma_start(out=xs[:, :, 128:256], in_=xB[:, :, 128:256])
            nc.sync.dma_start(out=xs[:, :, 256:512], in_=xB[:, :, 256:512])
            nc.sync.dma_start(out=xs[:, :, 512:1024], in_=xB[:, :, 512:1024])
            nc.sync.dma_start(out=xs[:, :, 1024:1536], in_=xB[:, :, 1024:1536])
            nc.sync.dma_start(out=xs[:, :, 1536:2048], in_=xB[:, :, 1536:2048])

            # dependency-free PE warmup (values irrelevant). Lives in the wap
            # pool (first real use ~6.5us) so qproj(0) never waits on it.
            warm = wap.tile([128, SB], f32, tag="wap", name="warm")
            for j in range(N_WARMUP):
                nc.tensor.transpose(
                    warm[0:64, 64 * (j % 8) : 64 * (j % 8) + 64],
                    warm_in, warm_in,
                )

            # tensor_scalar needs an f32 scalar operand: unpack the bias col
            wqbf = singles.tile([128, 1], f32, tag="wqbf")
            nc.vector.tensor_copy(wqbf, wq[:, 512:513])

            identf = singles.tile([128, 64], f32, tag="identf")
            nc.gpsimd.memset(identf, 1.0)
            for p0 in (0, 64):
                nc.gpsimd.affine_select(
                    out=identf[p0 : p0 + 64, :], in_=identf[p0 : p0 + 64, :],
                    compare_op=ALU.is_equal,
                    fill=0.0, base=0, pattern=[[-1, 64]], channel_multiplier=1,
                )

            ident64 = singles.tile([128, 64], bf16, tag="ident64")
            nc.vector.tensor_copy(ident64, identf)
            identb = singles.tile([128, 128], bf16, tag="identb")
            nc.gpsimd.memset(identb, 1.0)
            nc.gpsimd.affine_select(
                out=identb, in_=identb, compare_op=ALU.is_equal,
                fill=0.0, base=0, pattern=[[-1, 128]], channel_multiplier=1,
            )
            # trib[k, q] = 1 if k <= q else 0   (iota = q - k >= 0)
            trib = singles.tile([128, 128], bf16, tag="trib")
            nc.gpsimd.memset(trib, 1.0)
            nc.gpsimd.affine_select(
                out=trib, in_=trib, compare_op=ALU.is_ge,
                fill=0.0, base=0, pattern=[[1, 128]], channel_multiplier=-1,
            )

            qT = singles.tile([128, S], bf16, tag="qT")
            v_sb = [
                singles.tile([128, 16, 65], bf16, tag=f"v{h}", name=f"v{h}")
                for h in range(2)
            ]
            for h in range(2):
                nc.gpsimd.memset(v_sb[h][:, :, 64:65], 1.0)

            # SWDGE (Pool) carries woT: needed only by the first Wo (~12us).
            # Emitted after the Pool constants so its transfer doesn't jump
            # ahead of the early x chunks on the shared DMA pipe.
            woT = singles.tile([128, HID], bf16, tag="woT")
            nc.gpsimd.dma_start(out=woT, in_=WoT[:, :])

            # state shared across the emission helpers
            et_map = {}    # (h, qb, kc) -> (et_tile, col_of_qc0)
            pending_masks = {}  # (h, qb, qc_local) -> (et_tile, col)
            ao_tiles = {}  # (h, qc_local) -> ao tile (bf16 [128, 64])
            av_cur = {}    # h -> av psum tile
            aot_ps = {}    # (h, qb) -> psum tile [64, 4, 128] bf16
            aot_sb = {}    # qb -> sbuf tile [128, 4, 128] bf16

            # ---------------- emission helpers ----------------
            def qproj(qb, halves=False):
                s0 = qb * SB
                if halves:
                    # PSUM matmul WAR tracking is coarse: a part's matmuls
                    # wait on the previous parts' bias-add reads if they
                    # share a tile. Separate tiles/pools let the three
                    # DMA-gated parts of qb0 overlap.
                    parts = (
                        (0, 128, ppp.tile([128, 128], f32, tag="pp", name="qp0a")),
                        (128, 256, wap.tile([128, 128], f32, tag="wap", name="qp0b")),
                        (256, 512, ppp.tile([128, 256], f32, tag="pp", name="qp0c")),
                    )
                else:
                    parts = ((0, SB, ppp.tile([128, SB], f32, tag="pp", name=f"qp{qb}")),)
                for c0, c1, qp in parts:
                    for i in range(4):
                        if qb == 0 and c1 <= 128:
                            # cols 0:128 rode in with wq (one packed DMA)
                            xsrc = wqx[:, 513 + 128 * i : 513 + 128 * i + 128]
                        else:
                            xsrc = xs[:, i, s0 + c0 : s0 + c1]
                        nc.tensor.matmul(
                            qp[:, 0 : c1 - c0], lhsT=wq[:, 128 * i : 128 * i + 128],
                            rhs=xsrc,
                            start=(i == 0), stop=(i == 3),
                        )
                    nc.vector.tensor_scalar_add(
                        qT[:, s0 + c0 : s0 + c1], qp[:, 0 : c1 - c0], wqbf
                    )


            def vprep(h, qb):
                hp = 64 * h
                vt = ppp.tile([128, 4, 64], bf16, tag="pp", name=f"vt{h}_{qb}")
                for j in range(4):
                    t0 = 128 * (4 * qb + j)
                    nc.tensor.transpose(
                        vt[:, j, :], qT[hp : hp + 64, t0 : t0 + 128],
                        ident64[hp : hp + 64, :],
                    )
                nc.vector.tensor_copy(v_sb[h][:, 4 * qb : 4 * qb + 4, 0:64], vt)

            def qk_group(h, qb, chunks, expw, masks, exp_splits=None,
                         defer_masks=True, small=False):
                """chunks: [(kc, coff, qoff, N)]; masks: [col] of tri blocks.
                small=True puts the narrow diag-1 scores in a wap-pool bank:
                it leaves the qkp pingpong, so diag-2 and the next unit's
                first QK wait only long-finished exps."""
                hp = 64 * h
                s0 = qb * SB
                if small:
                    qk = wap.tile([128, SB], f32, tag="wap", name="qkS")
                else:
                    qk = qkp.tile([128, 1024], f32, tag="qk", name="qk")
                et = etp.tile([128, 1024], bf16, tag="et", name="et")
                ranges = exp_splits or [(0, expw)]
                for kc, coff, qoff, n in chunks:
                    t0 = 128 * kc
                    nc.tensor.matmul(
                        qk[:, coff : coff + n],
                        lhsT=qT[hp : hp + 64, t0 : t0 + 128],
                        rhs=qT[hp : hp + 64, s0 + qoff : s0 + qoff + n],
                        start=True, stop=True,
                    )
                    et_map[(h, qb, kc)] = (et, coff - 128 * (qoff // 128))
                    # exp as soon as the covering chunk(s) are in psum
                    while ranges and ranges[0][1] <= coff + n:
                        e0, e1 = ranges.pop(0)
                        nc.scalar.activation(
                            out=et[:, e0:e1], in_=qk[:, e0:e1],
                            func=Exp, scale=SCALE,
                        )
                for e0, e1 in ranges:
                    nc.scalar.activation(
                        out=et[:, e0:e1], in_=qk[:, e0:e1], func=Exp, scale=SCALE
                    )
                for qc_l, mc in masks:
                    if defer_masks:
                        pending_masks[(h, qb, qc_l)] = (et, mc)
                    else:
                        # final unit: Pool is idle and this keeps the DVE
                        # queue clear for the drain chains' recip/mul
                        nc.gpsimd.tensor_mul(
                            et[:, mc : mc + 128], et[:, mc : mc + 128], trib
                        )

            def unit_groups(h, qb, split_first=False, defer_masks=True,
                            diag_pos=None):
                k0 = 4 * qb
                gs = []
                for ke in range(0, k0, 2):  # off-diagonal pairs, full width
                    gs.append(
                        lambda ke=ke: qk_group(
                            h, qb,
                            [(ke, 0, 0, 512), (ke + 1, 512, 0, 512)],
                            1024, [],
                        )
                    )
                ch0 = (
                    [(k0, 0, 0, 128), (k0, 128, 128, 128), (k0, 256, 256, 256)]
                    if split_first else [(k0, 0, 0, 512)]
                )
                splits = [(0, 128), (128, 256), (256, 512)] if split_first else None
                diags = [
                    # diag pack A: kc0 alone (N=512). The NARROW exp comes
                    # first so the next unit's first QK (pingpong on this
                    # tile) unblocks early and overlaps pack B's wider exp:
                    # no ACT gap at unit boundaries.
                    lambda: qk_group(
                        h, qb, ch0, 512, [(0, 0)],
                        exp_splits=splits, defer_masks=defer_masks,
                        # the drain unit (diag_pos set) keeps qkp: no next
                        # unit to unblock, and its wap rotation is busy with
                        # d3's drain tiles
                        small=not split_first and diag_pos is None,
                    ),
                    # diag pack B: kc1 (384) + kc3 (128) + kc2 (256), packed
                    # 0..768 contiguously without crossing a psum bank
                    lambda: qk_group(
                        h, qb,
                        [(k0 + 1, 0, 128, 384), (k0 + 3, 384, 384, 128),
                         (k0 + 2, 512, 256, 256)],
                        768, [(1, 0), (2, 512), (3, 384)],
                        defer_masks=defer_masks,
                    ),
                ]
                if diag_pos is None:
                    return gs + diags
                return gs[:diag_pos] + diags + gs[diag_pos:]

            def av_item(h, qb, qc_local, kc_from=0, kc_to=None):
                qc = 4 * qb + qc_local
                if kc_to is None:
                    kc_to = qc + 1
                if qc_local == 0 and kc_from == 0:
                    av_cur[h] = avp.tile(
                        [128, 4, 65], f32, tag="av", name=f"av{h}{qb}"
                    )
                av = av_cur[h]
                if kc_to > qc:  # this call includes the diagonal chunk
                    pm = pending_masks.pop((h, qb, qc_local), None)
                    if pm is not None:
                        met, mc = pm
                        nc.vector.tensor_mul(
                            met[:, mc : mc + 128], met[:, mc : mc + 128], trib
                        )
                for kc in range(kc_from, min(kc_to, qc + 1)):
                    et, c0 = et_map[(h, qb, kc)]
                    nc.tensor.matmul(
                        av[:, qc_local, :],
                        lhsT=et[:, c0 + 128 * qc_local : c0 + 128 * qc_local + 128],
                        rhs=v_sb[h][:, kc, :],
                        start=(kc == 0), stop=(kc == qc),
                    )

            def av_pre_step(h, qb, kcs, stop_at_end=False):
                """kc-major partial AV accumulation for a drain block. PSUM
                allows ONE open accumulation group per 2KB zero region (the
                whole av tile), so the drain's entire AV is a single group:
                started here at kc==0, closed by av_diag(last=True) or by
                stop_at_end on the final chunk."""
                if kcs and kcs[0] == 0:
                    av_cur[h] = avp.tile(
                        [128, 4, 65], f32, tag="av", name=f"av{h}{qb}"
                    )
                av = av_cur[h]
                for kc in kcs:
                    et, c0 = et_map[(h, qb, kc)]
                    for qc_local in range(4):
                        nc.tensor.matmul(
                            av[:, qc_local, :],
                            lhsT=et[:, c0 + 128 * qc_local : c0 + 128 * qc_local + 128],
                            rhs=v_sb[h][:, kc, :],
                            start=(kc == 0 and qc_local == 0),
                            stop=(stop_at_end and kc == kcs[-1] and qc_local == 3),
                        )

            def av_diag(h, qb, qc_local, last):
                """Diagonal AV chunks of one drain chain; continues the open
                group from av_pre_step, stop only on the unit's final mm."""
                qc = 4 * qb + qc_local
                av = av_cur[h]
                pm = pending_masks.pop((h, qb, qc_local), None)
                if pm is not None:
                    met, mc = pm
                    nc.vector.tensor_mul(
                        met[:, mc : mc + 128], met[:, mc : mc + 128], trib
                    )
                for kc in range(4 * qb, qc + 1):
                    et, c0 = et_map[(h, qb, kc)]
                    nc.tensor.matmul(
                        av[:, qc_local, :],
                        lhsT=et[:, c0 + 128 * qc_local : c0 + 128 * qc_local + 128],
                        rhs=v_sb[h][:, kc, :],
                        start=False, stop=(last and kc == qc),
                    )

            def div_item(h, qc_local, rec=None):
                """ao = av[:, qc, 0:64] * (1 / av[:, qc, 64]) -> bf16."""
                av = av_cur[h]
                if rec is None:  # per-qc reciprocal (drain path)
                    rec = recp.tile([128, 1, 1], f32, tag="rec", name="rec1")
                    nc.vector.reciprocal(rec, av[:, qc_local, 64:65])
                    rslice = rec[:, 0, :]
                else:
                    rslice = rec[:, qc_local, :]
                ao = aop.tile([128, 64], bf16, tag="ao", name="ao")
                nc.vector.tensor_scalar_mul(ao, av[:, qc_local, 0:64], rslice)
                ao_tiles[(h, qc_local)] = ao

            def norm_item(h, qb):
                rec = recp.tile([128, 4, 1], f32, tag="rec", name="rec4")
                nc.vector.reciprocal(rec, av_cur[h][:, :, 64:65])
                # all 4 qc normalized in ONE DVE op via a stride-0 broadcast
                # of the per-(partition, qc) reciprocal
                ao4 = aop.tile([128, 4, 64], bf16, tag="ao4", name="ao4")
                a_ap, r_ap = bass.broadcast_tensor_aps(
                    av_cur[h][:, :, 0:64], rec[:, :, :]
                )
                nc.vector.tensor_tensor(out=ao4, in0=a_ap, in1=r_ap, op=ALU.mult)
                for qc_local in range(4):
                    ao_tiles[(h, qc_local)] = ao4[:, qc_local, :]

            def t_item(h, qb):
                ps = wap.tile([64, 4, 128], bf16, tag="wap", name=f"aot{h}{qb}")
                aot_ps[(h, qb)] = ps
                for qc_local in range(4):
                    nc.tensor.transpose(
                        ps[:, qc_local, :], ao_tiles[(h, qc_local)], identb
                    )

            def aot_copy(qb, h):
                if qb not in aot_sb:
                    aot_sb[qb] = aotsb.tile(
                        [128, 4, 128], bf16, tag="aotsb", name=f"aotsb{qb}"
                    )
                sb = aot_sb[qb]
                nc.vector.tensor_copy(
                    sb[64 * h : 64 * h + 64, :, :], aot_ps[(h, qb)]
                )

            def w_item(qb, qc_local, copy_eng, dma_eng, wp=None):
                if wp is None:
                    wp = wap.tile(
                        [128, SB], f32, tag="wap", name=f"wp{qb}{qc_local}"
                    )
                nc.tensor.matmul(
                    wp, lhsT=aot_sb[qb][:, qc_local, :], rhs=woT,
                    start=True, stop=True,
                )
                ob = obp.tile([128, SB], bf16, tag="ob", name="ob")
                if hasattr(copy_eng, "tensor_copy"):
                    copy_eng.tensor_copy(ob, wp)
                else:
                    copy_eng.copy(ob, wp)  # scalar engine (ACT)
                r0 = 512 * qb + 128 * qc_local
                dma_eng.dma_start(out=out_part[r0 : r0 + 128, :], in_=ob)

            def w_items(qb):
                # ob copies must read PSUM: only DVE/ACT can. DMAs alternate
                # between the HWDGE (sync) and SWDGE (gpsimd) queues.
                out = []
                for qc_local in range(4):
                    out.append(lambda q=qb, c=qc_local: w_item(q, c, nc.vector, nc.sync))
                return out

            def av_norm_t(h, qb):
                return [lambda c=c: av_item(h, qb, c) for c in range(4)] + [
                    lambda: norm_item(h, qb),
                    lambda: t_item(h, qb),
                ]

            def drain_steps(qb, engines, use_qkp=False, pre=False):
                """Software-pipelined drain of head 1 of block qb: the four
                per-qc chains (AV -> divide -> transpose -> copy -> Wo -> DMA)
                emitted as a diagonal wavefront so the in-order engines never
                wait a full chain. Returns a list of emit-thunks (steps); with
                pre=True the off-diagonal AV accumulation is split out as four
                leading steps that only need the unit's off-diagonal exps."""
                h = 1
                ps_t = {}

                def av_s(qc):
                    if not pre:
                        return lambda: av_item(h, qb, qc)
                    return lambda: av_diag(h, qb, qc, last=(qc == 3))

                def div_s(qc):
                    return lambda: div_item(h, qc)

                def t_s(qc):
                    def f():
                        ps = wap.tile(
                            [64, 1, 128], bf16, tag="wap", name=f"aotd{qb}{qc}"
                        )
                        ps_t[qc] = ps
                        nc.tensor.transpose(
                            ps[:, 0, :], ao_tiles[(h, qc)], identb
                        )
                    return f

                def cp_s(qc):
                    def f():
                        nc.vector.tensor_copy(
                            aot_sb[qb][64:128, qc, :], ps_t[qc][:, 0, :]
                        )
                    return f

                def w_s(qc, use_qkp):
                    ce, de = engines[qc]

                    def f():
                        wp = None
                        if use_qkp:
                            # the QK stream is done: its psum banks are free.
                            # One tile per Wo — sharing a [128,1024] tile
                            # between two Wos serializes the second behind
                            # the first's output copy (coarse matmul WAR).
                            wp = qkp.tile(
                                [128, 1024], f32, tag="qk", name=f"wpd{qc}"
                            )[:, 0:512]
                        w_item(qb, qc, ce, de, wp=wp)

                    return f

                uq = use_qkp
                if pre:
                    # batched variant: diag AVs, then ONE broadcast norm for
                    # all 4 qc, then transposes/copies/Wos wavefronted
                    waves = [
                        [av_s(0), av_s(1)],
                        [av_s(2), av_s(3)],
                        [lambda: norm_item(h, qb)],
                        [t_s(0), t_s(1)],
                        [t_s(2), t_s(3), cp_s(0)],
                        [cp_s(1), cp_s(2), cp_s(3)],
                        [w_s(0, uq), w_s(1, uq)],
                        [w_s(2, uq), w_s(3, uq)],
                    ]
                else:
                    waves = [
                        [av_s(0)],
                        [av_s(1), div_s(0)],
                        [av_s(2), div_s(1), t_s(0)],
                        [av_s(3), div_s(2), cp_s(0), t_s(1)],
                        [div_s(3), cp_s(1), t_s(2), w_s(0, uq)],
                        [cp_s(2), t_s(3), w_s(1, uq)],
                        [cp_s(3), w_s(2, uq)],
                        [w_s(3, uq)],
                    ]

                def run(wave):
                    return lambda: [f() for f in wave]

                steps = [run(w) for w in waves]
                if pre:
                    nk = 4 * qb
                    chunks = [
                        list(range(nk * i // 4, nk * (i + 1) // 4))
                        for i in range(4)
                    ]
                    steps = [
                        lambda ks=ks: av_pre_step(h, qb, ks) for ks in chunks
                    ] + steps
                return steps

            def emit_unit(h, qb, hk, split_first=False, defer_masks=True,
                          diag_pos=None, skip_first=0):
                gs = unit_groups(h, qb, split_first=split_first,
                                 defer_masks=defer_masks, diag_pos=diag_pos)
                gs = gs[skip_first:]
                hk = list(hk)
                for g in gs:
                    g()
                    if hk:
                        hk.pop(0)()
                for item in hk:
                    item()
                return []


            def interleave(ws, avs, rest):
                """Alternate stall-prone W chains with cheap AV filler so an
                in-order PE never has two wp-waits back to back."""
                out = []
                for i in range(max(len(ws), len(avs))):
                    if i < len(ws):
                        out.append(ws[i])
                    if i < len(avs):
                        out.append(avs[i])
                return out + rest

            # ---------------- main schedule ----------------
            # unit order: (0,0) (1,0) (0,1) (1,1) (0,2) (0,3) (1,3)
            #             (1,2)+[qb3 drain] [qb2 drain]
            qproj(0, halves=True)
            emit_unit(
                0, 0, [lambda: vprep(0, 0), lambda: vprep(1, 0)],
                split_first=True,
            )
            emit_unit(
                1, 0,
                [lambda: qproj(1)] + av_norm_t(0, 0)
                + [lambda: aot_copy(0, 0)],
            )
            emit_unit(
                0, 1,
                av_norm_t(1, 0)
                + [lambda: aot_copy(0, 1),
                   lambda: vprep(0, 1), lambda: vprep(1, 1)],
            )
            emit_unit(
                1, 1,
                [lambda: qproj(2)]
                + interleave(
                    w_items(0),
                    [lambda c=c: av_item(0, 1, c) for c in range(4)],
                    [lambda: norm_item(0, 1), lambda: t_item(0, 1),
                     lambda: aot_copy(1, 0)],
                ),
            )
            emit_unit(
                0, 2,
                [lambda: qproj(3)] + av_norm_t(1, 1)
                + [lambda: aot_copy(1, 1),
                   lambda: vprep(0, 2), lambda: vprep(1, 2)],
            )
            emit_unit(
                0, 3,
                interleave(
                    w_items(1),
                    [lambda c=c: av_item(0, 2, c) for c in range(4)],
                    [lambda: norm_item(0, 2), lambda: t_item(0, 2),
                     lambda: aot_copy(2, 0),
                     lambda: vprep(0, 3), lambda: vprep(1, 3)],
                ),
            )
            # qb3 drain: off-diag AV pre-accumulates inside unit (1,3); the
            # per-qc chains (diag AV -> div -> t -> cp -> Wo -> ob -> DMA)
            # ride unit (1,2)'s off-diag groups. Output DMAs go out SWDGE
            # (Pool gen) so the HWDGE unit stays clear for the final drain.
            d3 = drain_steps(3, {c: (nc.vector, nc.gpsimd) for c in range(4)},
                             pre=True)
            emit_unit(
                1, 3,
                av_norm_t(0, 3) + [lambda: aot_copy(3, 0)] + d3[:4],
            )
            # qb2 drain: unit (1,2) runs with its diagonal packs mid-unit
            # (off01, off23, diagA, diagB, off45, off67) so the masks and
            # diag AV land mid-stream. The single AV accumulation group
            # closes on the last off-diag chunk right at stream end, so all
            # four chains launch together.
            w3 = d3[4:]
            pair = lambda a, b: (lambda: (a(), b()))

            def d2_masks():
                # pop + apply all four masks ahead of d3's final ob copies in
                # the DVE queue, so the diag AVs don't stall on a 658ns copy
                for qc in range(4):
                    pm = pending_masks.pop((1, 2, qc), None)
                    if pm is not None:
                        met, mc = pm
                        nc.vector.tensor_mul(
                            met[:, mc : mc + 128], met[:, mc : mc + 128], trib
                        )

            emit_unit(
                1, 2,
                [pair(w3[0], w3[1]), pair(w3[2], w3[3]),
                 pair(w3[4], w3[5]),
                 pair(d2_masks, pair(w3[6], w3[7])),
                 # av12 allocation must follow ALL d3 wave emissions (avp is
                 # a single rotating bank shared with av13)
                 pair(lambda: av_pre_step(1, 2, [0, 1, 2, 3, 4, 5]),
                      lambda: [av_diag(1, 2, c, last=False) for c in range(4)]),
                 lambda: av_pre_step(1, 2, [6, 7], stop_at_end=True)],
                diag_pos=2,
            )
            # post-stream chains: one reciprocal for all 4 qc; aot copies
            # split DVE/ACT; Wo psum spread over four pools (no bank reuse
            # stalls); the last two outputs DMA straight from psum as f32
            # (host converts), the first two copy to bf16 on ACT/DVE.
            rec4 = recp.tile([128, 4, 1], f32, tag="rec", name="rec4d")
            nc.vector.reciprocal(rec4, av_cur[1][:, :, 64:65])
            ao4d = aop.tile([128, 4, 64], bf16, tag="ao4", name="ao4d")
            a_ap, r_ap = bass.broadcast_tensor_aps(
                av_cur[1][:, :, 0:64], rec4[:, :, :]
            )
            nc.vector.tensor_tensor(out=ao4d, in0=a_ap, in1=r_ap, op=ALU.mult)
            ps_t2 = {}
            for qc in range(4):
                ps = wap.tile([64, 1, 128], bf16, tag="wap", name=f"aotd2{qc}")
                ps_t2[qc] = ps
                nc.tensor.transpose(ps[:, 0, :], ao4d[:, qc, :], identb)
            for qc, ce in ((0, nc.vector), (1, nc.scalar),
                           (2, nc.vector), (3, nc.scalar)):
                if hasattr(ce, "tensor_copy"):
                    ce.tensor_copy(aot_sb[2][64:128, qc, :], ps_t2[qc][:, 0, :])
                else:
                    ce.copy(aot_sb[2][64:128, qc, :], ps_t2[qc][:, 0, :])
            wps = {}
            for qc in range(4):
                if qc < 2:
                    wp = qkp.tile([128, 1024], f32, tag="qk",
                                  name=f"wpd{qc}")[:, 0:512]
                elif qc == 2:
                    wp = ppp.tile([128, SB], f32, tag="pp", name="wpd2")
                else:
                    wp = wap.tile([128, SB], f32, tag="wap", name="wpd3")
                nc.tensor.matmul(wp, lhsT=aot_sb[2][:, qc, :], rhs=woT,
                                 start=True, stop=True)
                wps[qc] = wp
            # output copies: qc0 full on ACT, qc1 full on DVE; the two LAST
            # chains split their copies half/half across ACT+DVE (~350ns
            # each) so the final DMA launches ~0.6us sooner. Early outputs
            # ride SWDGE (Pool gen, parallel to the HWDGE unit).
            for qc, ce, de in ((0, nc.scalar, nc.gpsimd),
                               (1, nc.vector, nc.gpsimd),
                               (2, nc.scalar, nc.sync),
                               (3, nc.vector, nc.sync)):
                ob = obp.tile([128, SB], bf16, tag="ob", name=f"obd{qc}")
                if hasattr(ce, "tensor_copy"):
                    ce.tensor_copy(ob, wps[qc])
                else:
                    ce.copy(ob, wps[qc])
                r0 = 1024 + 128 * qc
                de.dma_start(out=out_part[r0 : r0 + 128, :], in_=ob)

    nc.finalize()
    return nc


_NC_CACHE = None


def _get_nc():
    global _NC_CACHE
    if _NC_CACHE is None:
        _NC_CACHE = build_nc()
    return _NC_CACHE


def make_in_maps(x, Wq_w, Wq_b, Wo_w):
    x = np.asarray(x, dtype=np.float32)
    Wq_w = np.asarray(Wq_w, dtype=np.float32)
    Wq_b = np.asarray(Wq_b, dtype=np.float32)
    Wo_w = np.asarray(Wo_w, dtype=np.float32)
    in_maps = []
    for c in range(N_CORES):
        b, hp = divmod(c, 4)
        dq = slice(128 * hp, 128 * (hp + 1))
        xBc = np.ascontiguousarray(x[b].T.reshape(4, 128, S).transpose(1, 0, 2))
        WqBc = np.ascontiguousarray(
            Wq_w[dq, :].T.reshape(4, 128, 128).transpose(1, 0, 2)
        )
        WqBp = np.concatenate(
            [
                WqBc.reshape(128, 512),
                Wq_b[dq].reshape(128, 1),
                # first 128 query columns of each i-block ride with wq
                xBc[:, :, 0:128].reshape(128, 512),
            ],
            axis=1,
        )
        in_maps.append({
            "xB": xBc.astype(ml_dtypes.bfloat16),
            "WqX": np.ascontiguousarray(WqBp).astype(ml_dtypes.bfloat16),
            "WoT": np.ascontiguousarray(Wo_w[:, dq].T).astype(ml_dtypes.bfloat16),
        })
    return in_maps


def kernel(x, mask, Wq_w, Wq_b, Wo_w, Wo_b, **_):
    nc = _get_nc()
    in_maps = make_in_maps(x, Wq_w, Wq_b, Wo_w)
    res = run_bass_kernel_spmd(nc, in_maps, core_ids=list(range(N_CORES)))
    Wo_b = np.asarray(Wo_b, dtype=np.float32)
    out = np.empty((B, S, HID), dtype=np.float32)
    for b in range(B):
        acc = np.asarray(res.results[4 * b]["out_part"], dtype=np.float32)
        for c in range(4 * b + 1, 4 * b + 4):
            acc = acc + np.asarray(res.results[c]["out_part"], dtype=np.float32)
        out[b] = acc + Wo_b[None, :]
    return out

